# revision 6
# baseline (speedup 1.0000x reference)
"""Causal self-attention (B=2, T=2048, E=2048, H=16, D=128) on 8 NeuronCores.

Sharding: tensor-parallel over heads — each core owns 2 heads (256 features).
Per core: QKV projections for its head slice (fp32r matmuls), RoPE, causal
attention computed in S^T layout (keys on partitions), and a partial output
projection against its Wo row-slice. The host sums the 8 partials and adds bo.

All matmul operands are fp32r (fp32 rounded to 11 mantissa bits) — full PE
rate for moving free-dim >= 256, near-fp32 accuracy.
"""
import sys

sys.path.insert(0, "/opt/trn_rl_repo")

import numpy as np

import concourse.mybir as mybir
import concourse.tile as tile
from concourse import bacc
from concourse.bass_utils import run_bass_kernel_spmd

B, T, E, H = 2, 2048, 2048, 16
D = E // H            # 128 head dim
N_CORES = 8
HPC = H // N_CORES    # 2 heads per core
FPC = HPC * D         # 256 features per core
ROPE_BASE = 10000.0

CH = 512              # t-chunk (moving free dim)
NCH = T // CH         # 4 chunks
KT = E // 128         # 16 contraction tiles
NTT = T // 128        # 16 t-subtiles

f32 = mybir.dt.float32
f32r = mybir.dt.float32r


def round_fp32r(x: np.ndarray) -> np.ndarray:
    """Round fp32 to the fp32r representation (RNE to 11 mantissa bits)."""
    b = np.ascontiguousarray(x, np.float32).view(np.uint32).astype(np.uint64)
    r = b + 0x7FF + ((b >> 12) & 1)
    r = (r & ~np.uint64(0xFFF)).astype(np.uint32)
    return r.view(np.float32)


def build_nc(reps: int = 1):
    """Build the per-core Bass program. reps>1 wraps the body in a hardware
    repeat loop (identical work each iteration) for slope-timing."""
    nc = bacc.Bacc("TRN2", target_bir_lowering=False, debug=False,
                   num_devices=N_CORES)

    xT = nc.dram_tensor("xT", [B, E, T], f32r, kind="ExternalInput")
    wqkvT = nc.dram_tensor("wqkvT", [E, 3 * FPC], f32r, kind="ExternalInput")
    woT = nc.dram_tensor("woT", [FPC, E], f32r, kind="ExternalInput")
    tabs = nc.dram_tensor("tabs", [4, D, T], f32, kind="ExternalInput")
    masks = nc.dram_tensor("masks", [4, 128, CH], f32r, kind="ExternalInput")
    bqk = nc.dram_tensor("bqk", [128, 2 * HPC], f32, kind="ExternalInput")
    bvT = nc.dram_tensor("bvT", [1, FPC], f32r, kind="ExternalInput")
    ones_row = nc.dram_tensor("ones_row", [1, 128], f32r, kind="ExternalInput")
    ones_col = nc.dram_tensor("ones_col", [128, 1], f32r, kind="ExternalInput")
    rotm = nc.dram_tensor("rotm", [D, D], f32r, kind="ExternalInput")
    y = nc.dram_tensor("y", [B, T, E], f32, kind="ExternalOutput")

    Exp = mybir.ActivationFunctionType.Exp
    Identity = mybir.ActivationFunctionType.Identity
    Mult = mybir.AluOpType.mult
    Add = mybir.AluOpType.add

    with tile.TileContext(nc) as tc:
        with (
            nc.allow_low_precision(reason="fp32r matmul operands are intentional"),
            tc.tile_pool(name="wpool", bufs=1) as wpool,
            tc.tile_pool(name="xc", bufs=2) as xcp,
            tc.tile_pool(name="qkv", bufs=1) as qkvp,
            tc.tile_pool(name="tab", bufs=1) as tabp,
            tc.tile_pool(name="const", bufs=1) as constp,
            tc.tile_pool(name="pt", bufs=3) as ptp,
            tc.tile_pool(name="pacc", bufs=2) as paccp,
            tc.tile_pool(name="ot", bufs=1) as otp,
            tc.tile_pool(name="ybuf", bufs=2) as ybufp,
            tc.tile_pool(name="rope", bufs=2) as ropep,
            tc.tile_pool(name="small", bufs=1) as smallp,
            tc.tile_pool(name="psb", bufs=6, space="PSUM") as psb,
            tc.tile_pool(name="pss", bufs=2, space="PSUM") as pss,
        ):
            # ---- resident constants (loaded once) ----
            w_sb = wpool.tile([128, KT, 3 * FPC], f32r, tag="wqkv")
            nc.sync.dma_start(
                out=w_sb[:, :, :],
                in_=wqkvT.ap().rearrange("(kt p) f -> p kt f", p=128))
            mask_sb = constp.tile([128, 4, CH], f32r, tag="masks")
            nc.sync.dma_start(out=mask_sb[:, :, :], in_=masks.ap().rearrange("m p q -> p m q"))
            bqk_sb = constp.tile([128, 2 * HPC], f32, tag="bqk")
            nc.sync.dma_start(out=bqk_sb[:, :], in_=bqk.ap())
            bv_sb = constp.tile([1, FPC], f32r, tag="bv")
            nc.sync.dma_start(out=bv_sb[:, :], in_=bvT.ap())
            onesr_sb = constp.tile([1, 128], f32r, tag="onesr")
            nc.sync.dma_start(out=onesr_sb[:, :], in_=ones_row.ap())
            onesc_sb = constp.tile([128, 1], f32r, tag="onesc")
            nc.sync.dma_start(out=onesc_sb[:, :], in_=ones_col.ap())
            rot_sb = constp.tile([D, D], f32r, tag="rotm")
            nc.sync.dma_start(out=rot_sb[:, :], in_=rotm.ap())

            def body():
                for b in range(B):
                    # ================= stage A: projections + RoPE =========
                    qt_sb = [qkvp.tile([128, T], f32r, tag=f"qt{h}", name=f"qt{h}") for h in range(HPC)]
                    kt_sb = [qkvp.tile([128, T], f32r, tag=f"kt{h}", name=f"ktt{h}") for h in range(HPC)]
                    v_sb = qkvp.tile([128, NTT, FPC], f32r, tag="v")

                    for c in range(NCH):
                        # x chunk (transposed layout), two half-loads of 8 e-tiles
                        xh = []
                        for half in range(2):
                            t_ = xcp.tile([128, 8, CH], f32r, tag="xc")
                            src = xT.ap()[b][:, c * CH:(c + 1) * CH].rearrange(
                                "(et p) t -> p et t", p=128)
                            nc.sync.dma_start(out=t_[:, :, :],
                                              in_=src[:, half * 8:half * 8 + 8, :])
                            xh.append(t_)

                        # rope table slices: 0 cosQ, 1 sinQ, 2 cosK, 3 sinK
                        tsl = []
                        for ti in range(4):
                            tt = tabp.tile([128, CH], f32, tag=f"tab{ti}")
                            nc.sync.dma_start(
                                out=tt[:, :],
                                in_=tabs.ap()[ti][:, c * CH:(c + 1) * CH])
                            tsl.append(tt)

                        # Q^T and K^T projections + RoPE per head
                        for which, dst in ((0, qt_sb), (1, kt_sb)):
                            for h in range(HPC):
                                fofs = which * FPC + h * D
                                ps = psb.tile([128, CH], f32, tag="big")
                                for k in range(KT):
                                    nc.tensor.matmul(
                                        ps[:, :],
                                        w_sb[:, k, fofs:fofs + D],
                                        xh[k // 8][:, k % 8, :],
                                        start=(k == 0), stop=(k == KT - 1))
                                # bias add (per-partition) on ACT, PSUM -> SBUF
                                qb = ropep.tile([128, CH], f32r, tag="qb")
                                col = which * HPC + h
                                nc.scalar.activation(qb[:, :], ps[:, :], Identity,
                                                     bias=bqk_sb[:, col:col + 1])
                                # rot(q) via +-1 permutation matrix on the PE
                                qr_ps = psb.tile([128, CH], f32, tag="big")
                                nc.tensor.matmul(qr_ps[:, :], rot_sb[:, :], qb[:, :],
                                                 start=True, stop=True)
                                # RoPE: out = qb*cos + rot(qb)*sin
                                ct, st = tsl[2 * which], tsl[2 * which + 1]
                                t1 = ropep.tile([128, CH], f32, tag="t1", bufs=1)
                                nc.vector.tensor_mul(t1[:, :], qb[:, :], ct[:, :])
                                t2 = ropep.tile([128, CH], f32, tag="t2", bufs=1)
                                nc.vector.tensor_mul(t2[:, :], qr_ps[:, :], st[:, :])
                                out = dst[h]
                                cc = slice(c * CH, (c + 1) * CH)
                                nc.vector.tensor_add(out[:, cc], t1[:, :], t2[:, :])

                        # V projection (natural layout), x tiles stationary
                        for tsub in range(4):
                            tt = c * 4 + tsub
                            ps = psb.tile([128, FPC], f32, tag="big")
                            for k in range(KT):
                                nc.tensor.matmul(
                                    ps[:, :],
                                    xh[k // 8][:, k % 8, tsub * 128:tsub * 128 + 128],
                                    w_sb[:, k, 2 * FPC:3 * FPC],
                                    start=(k == 0), stop=False)
                            # bias via K=1 ones matmul
                            nc.tensor.matmul(ps[:, :], onesr_sb[:, :], bv_sb[:, :],
                                             start=False, stop=True)
                            nc.vector.tensor_copy(v_sb[:, tt, :], ps[:, :])

                    # ================= stage B: attention ==================
                    ot_sb = [otp.tile([128, T], f32r, tag=f"ot{h}", name=f"ot{h}") for h in range(HPC)]
                    for h in range(HPC):
                        for c in range(NCH):
                            njt = 4 * c + 4   # causal k-tiles for this q-chunk
                            o_ps = psb.tile([128, CH], f32, tag="big")
                            pacc = paccp.tile([128, CH], f32r, tag="pacc")
                            cc = slice(c * CH, (c + 1) * CH)
                            for j in range(njt):
                                s_ps = psb.tile([128, CH], f32, tag="big")
                                nc.tensor.matmul(
                                    s_ps[:, :],
                                    kt_sb[h][:, j * 128:j * 128 + 128],
                                    qt_sb[h][:, cc],
                                    start=True, stop=True)
                                pt = ptp.tile([128, CH], f32r, tag="pt")
                                nc.scalar.activation(pt[:, :], s_ps[:, :], Exp)
                                if j >= 4 * c:
                                    nc.vector.tensor_mul(
                                        pt[:, :], pt[:, :],
                                        mask_sb[:, j - 4 * c, :])
                                if j == 0:
                                    nc.vector.tensor_copy(pacc[:, :], pt[:, :])
                                else:
                                    nc.vector.tensor_add(pacc[:, :], pacc[:, :], pt[:, :])
                                nc.tensor.matmul(
                                    o_ps[:, :],
                                    v_sb[:, j, h * D:h * D + D],
                                    pt[:, :],
                                    start=(j == 0), stop=(j == njt - 1),
                                    skip_group_check=True)
                            # softmax denominator: partition-sum, recip+newton
                            rs = pss.tile([1, CH], f32, tag="rs")
                            nc.tensor.matmul(rs[:, :], onesc_sb[:, :], pacc[:, :],
                                             start=True, stop=True)
                            r0 = smallp.tile([1, CH], f32, tag="r0")
                            nc.vector.reciprocal(r0[:, :], rs[:, :])
                            tn = smallp.tile([1, CH], f32, tag="tn")
                            nc.vector.tensor_mul(tn[:, :], rs[:, :], r0[:, :])
                            nc.vector.tensor_scalar(tn[:, :], tn[:, :], -1.0, 2.0,
                                                    Mult, Add)
                            r1 = smallp.tile([1, CH], f32r, tag="r1")
                            nc.vector.tensor_mul(r1[:, :], r0[:, :], tn[:, :])
                            bc_ps = psb.tile([128, CH], f32, tag="big")
                            nc.tensor.matmul(bc_ps[:, :], onesr_sb[:, :], r1[:, :],
                                             start=True, stop=True)
                            bc_sb = ybufp.tile([128, CH], f32, tag="bc")
                            nc.scalar.activation(bc_sb[:, :], bc_ps[:, :], Identity)
                            nc.vector.tensor_mul(ot_sb[h][:, cc], o_ps[:, :], bc_sb[:, :])

                    # ================= stage C: output projection ==========
                    wo_sb = xcp.tile([128, HPC, E], f32r, tag="xc")
                    nc.sync.dma_start(
                        out=wo_sb[:, :, :],
                        in_=woT.ap().rearrange("(ft p) g -> p ft g", p=128))
                    for ti in range(NTT):
                        for gc in range(NCH):
                            yp = psb.tile([128, CH], f32, tag="big")
                            for h in range(HPC):
                                nc.tensor.matmul(
                                    yp[:, :],
                                    ot_sb[h][:, ti * 128:ti * 128 + 128],
                                    wo_sb[:, h, gc * CH:(gc + 1) * CH],
                                    start=(h == 0), stop=(h == HPC - 1))
                            yb = ybufp.tile([128, CH], f32, tag="yb")
                            nc.scalar.activation(yb[:, :], yp[:, :], Identity)
                            nc.sync.dma_start(
                                out=y.ap()[b][ti * 128:ti * 128 + 128,
                                              gc * CH:(gc + 1) * CH],
                                in_=yb[:, :])

            if reps == 1:
                body()
            else:
                with tc.For_i(0, reps, 1):
                    body()

    nc.compile()
    return nc


def host_inputs(x, Wq, bq, Wk, bk, Wv, bv, Wo, bo):
    """Prepare per-core input maps from the full problem inputs."""
    x = np.asarray(x, np.float32)
    xTr = round_fp32r(np.ascontiguousarray(x.transpose(0, 2, 1)))

    # RoPE tables, 1-indexed positions, 1/sqrt(D) folded into the Q tables
    j = np.arange(D // 2, dtype=np.float64)
    thetas = ROPE_BASE ** (-2.0 * j / D)
    m = np.arange(1, T + 1, dtype=np.float64)
    ang = m[:, None] * thetas[None, :]          # [T, D/2]
    ang = np.concatenate([ang, ang], axis=1)    # [T, D]
    s = 1.0 / np.sqrt(D)
    tabs = np.stack([
        (np.cos(ang) * s).T, (np.sin(ang) * s).T,
        np.cos(ang).T, np.sin(ang).T,
    ]).astype(np.float32)                        # [4, D, T]

    # causal masks for the 4 diagonal alignments: mask_p[kk, qq] = qq >= 128p + kk
    kk = np.arange(128)[:, None]
    qq = np.arange(CH)[None, :]
    masks = np.stack([(qq >= 128 * p + kk) for p in range(4)]).astype(np.float32)

    onesr = np.ones((1, 128), np.float32)
    onesc = np.ones((128, 1), np.float32)
    rotm = np.zeros((D, D), np.float32)
    for d in range(D // 2):
        rotm[d + D // 2, d] = -1.0   # qrot[d] = -q[d+64]
        rotm[d, d + D // 2] = 1.0    # qrot[d+64] = q[d]

    in_maps = []
    for c in range(N_CORES):
        fs = slice(c * FPC, (c + 1) * FPC)
        wqkvT = np.concatenate([Wq[fs].T, Wk[fs].T, Wv[fs].T], axis=1)  # [E, 768]
        woT = np.ascontiguousarray(Wo[:, fs].T)                        # [256, E]
        bqk_cols = np.stack([
            bq[fs][:D], bq[fs][D:], bk[fs][:D], bk[fs][D:],
        ], axis=1).astype(np.float32)                                  # [128, 4]
        in_maps.append({
            "xT": xTr,
            "wqkvT": round_fp32r(np.ascontiguousarray(wqkvT)),
            "woT": round_fp32r(woT),
            "tabs": tabs,
            "masks": masks,
            "bqk": bqk_cols,
            "bvT": round_fp32r(np.asarray(bv[fs], np.float32)[None, :]),
            "ones_row": onesr,
            "ones_col": onesc,
            "rotm": rotm,
        })
    return in_maps


_NC_CACHE = {}


def get_nc(reps: int = 1):
    if reps not in _NC_CACHE:
        _NC_CACHE[reps] = build_nc(reps)
    return _NC_CACHE[reps]


def kernel(x, Wq, bq, Wk, bk, Wv, bv, Wo, bo):
    in_maps = host_inputs(x, Wq, bq, Wk, bk, Wv, bv, Wo, bo)
    nc = get_nc(1)
    res = run_bass_kernel_spmd(nc, in_maps, list(range(N_CORES)))
    out = np.zeros((B, T, E), np.float64)
    for c in range(N_CORES):
        out += res.results[c]["y"].astype(np.float64)
    out += np.asarray(bo, np.float64)[None, None, :]
    return out.astype(np.float32)


# revision 11
# speedup vs baseline: 1.6409x; 1.6409x over previous
"""Causal self-attention (B=2, T=2048, E=2048, H=16, D=128) on 8 NeuronCores.

Sharding: tensor-parallel over heads — each core owns 2 heads (256 features).
Per core: QKV projections for its head slice (fp32r matmuls), RoPE, causal
attention computed in S^T layout (keys on partitions), and a partial output
projection against its Wo row-slice. The host sums the 8 partials and adds bo.

All matmul operands are fp32r (fp32 rounded to 11 mantissa bits) — full PE
rate for moving free-dim >= 256, near-fp32 accuracy.
"""
import sys

sys.path.insert(0, "/opt/trn_rl_repo")

import numpy as np

import concourse.mybir as mybir
import concourse.tile as tile
from concourse import bacc
from concourse.bass_utils import run_bass_kernel_spmd

B, T, E, H = 2, 2048, 2048, 16
D = E // H            # 128 head dim
N_CORES = 8
HPC = H // N_CORES    # 2 heads per core
FPC = HPC * D         # 256 features per core
ROPE_BASE = 10000.0

CH = 512              # t-chunk (moving free dim)
NCH = T // CH         # 4 chunks
KT = E // 128         # 16 contraction tiles
NTT = T // 128        # 16 t-subtiles

f32 = mybir.dt.float32
f32r = mybir.dt.float32r


def round_fp32r(x: np.ndarray) -> np.ndarray:
    """Round fp32 to the fp32r representation (RNE to 11 mantissa bits)."""
    b = np.ascontiguousarray(x, np.float32).view(np.uint32).astype(np.uint64)
    r = b + 0x7FF + ((b >> 12) & 1)
    r = (r & ~np.uint64(0xFFF)).astype(np.uint32)
    return r.view(np.float32)


def build_nc(reps: int = 1):
    """Build the per-core Bass program. reps>1 wraps the body in a hardware
    repeat loop (identical work each iteration) for slope-timing."""
    nc = bacc.Bacc("TRN2", target_bir_lowering=False, debug=False,
                   num_devices=N_CORES)

    xT = nc.dram_tensor("xT", [B, E, T], f32r, kind="ExternalInput")
    wqkvT = nc.dram_tensor("wqkvT", [E, 3 * FPC], f32r, kind="ExternalInput")
    woT = nc.dram_tensor("woT", [FPC, E], f32r, kind="ExternalInput")
    tabs = nc.dram_tensor("tabs", [4, D, T], f32, kind="ExternalInput")
    masks = nc.dram_tensor("masks", [4, 128, CH], f32r, kind="ExternalInput")
    bqk = nc.dram_tensor("bqk", [128, 2 * HPC], f32, kind="ExternalInput")
    bvT = nc.dram_tensor("bvT", [1, FPC], f32r, kind="ExternalInput")
    ones_row = nc.dram_tensor("ones_row", [1, 128], f32r, kind="ExternalInput")
    ones_col = nc.dram_tensor("ones_col", [128, 1], f32r, kind="ExternalInput")
    rotm = nc.dram_tensor("rotm", [D, D], f32r, kind="ExternalInput")
    y = nc.dram_tensor("y", [B, T, E], f32, kind="ExternalOutput")

    Exp = mybir.ActivationFunctionType.Exp
    Identity = mybir.ActivationFunctionType.Identity
    Mult = mybir.AluOpType.mult
    Add = mybir.AluOpType.add

    with tile.TileContext(nc) as tc:
        with (
            nc.allow_low_precision(reason="fp32r matmul operands are intentional"),
            tc.tile_pool(name="wpool", bufs=1) as wpool,
            tc.tile_pool(name="xc", bufs=2) as xcp,
            tc.tile_pool(name="qkv", bufs=1) as qkvp,
            tc.tile_pool(name="tab", bufs=1) as tabp,
            tc.tile_pool(name="const", bufs=1) as constp,
            tc.tile_pool(name="pt", bufs=4) as ptp,
            tc.tile_pool(name="ot", bufs=1) as otp,
            tc.tile_pool(name="ybuf", bufs=2) as ybufp,
            tc.tile_pool(name="rope", bufs=2) as ropep,
            tc.tile_pool(name="small", bufs=1) as smallp,
            tc.tile_pool(name="psb", bufs=6, space="PSUM") as psb,
            tc.tile_pool(name="pss", bufs=2, space="PSUM") as pss,
        ):
            # ---- resident constants (loaded once) ----
            w_sb = wpool.tile([128, KT, 3 * FPC], f32r, tag="wqkv")
            nc.sync.dma_start(
                out=w_sb[:, :, :],
                in_=wqkvT.ap().rearrange("(kt p) f -> p kt f", p=128))
            mask_sb = constp.tile([128, 4, CH], f32r, tag="masks")
            nc.sync.dma_start(out=mask_sb[:, :, :], in_=masks.ap().rearrange("m p q -> p m q"))
            bqk_sb = constp.tile([128, 2 * HPC], f32, tag="bqk")
            nc.sync.dma_start(out=bqk_sb[:, :], in_=bqk.ap())
            bv_sb = constp.tile([1, FPC], f32r, tag="bv")
            nc.sync.dma_start(out=bv_sb[:, :], in_=bvT.ap())
            onesr_sb = constp.tile([1, 128], f32r, tag="onesr")
            nc.sync.dma_start(out=onesr_sb[:, :], in_=ones_row.ap())
            onesc_sb = constp.tile([128, 1], f32r, tag="onesc")
            nc.sync.dma_start(out=onesc_sb[:, :], in_=ones_col.ap())
            rot_sb = constp.tile([D, D], f32r, tag="rotm")
            nc.sync.dma_start(out=rot_sb[:, :], in_=rotm.ap())

            def body():
                for b in range(B):
                    kt_sb = [qkvp.tile([128, T], f32r, tag=f"kt{h}", name=f"ktt{h}") for h in range(HPC)]
                    v_sb = qkvp.tile([128, NTT, FPC], f32r, tag="v")
                    wo_sb = wpool.tile([128, HPC, E], f32r, tag="wo")
                    nc.sync.dma_start(
                        out=wo_sb[:, :, :],
                        in_=woT.ap().rearrange("(ft p) g -> p ft g", p=128))

                    for c in range(NCH):
                        # ---- stage A: projections for chunk c ----
                        qt_sb = [qkvp.tile([128, CH], f32r, tag=f"qt{h}", name=f"qt{h}",
                                           bufs=2) for h in range(HPC)]
                        xh = []
                        for half in range(2):
                            t_ = xcp.tile([128, 8, CH], f32r, tag="xc")
                            src_ = xT.ap()[b][:, c * CH:(c + 1) * CH].rearrange(
                                "(et p) t -> p et t", p=128)
                            nc.sync.dma_start(out=t_[:, :, :],
                                              in_=src_[:, half * 8:half * 8 + 8, :])
                            xh.append(t_)
                        tsl = []
                        for ti in range(4):
                            tt = tabp.tile([128, CH], f32, tag=f"tab{ti}")
                            nc.sync.dma_start(
                                out=tt[:, :],
                                in_=tabs.ap()[ti][:, c * CH:(c + 1) * CH])
                            tsl.append(tt)

                        for which, dst in ((0, qt_sb), (1, kt_sb)):
                            for h in range(HPC):
                                fofs = which * FPC + h * D
                                ps = psb.tile([128, CH], f32, tag="big")
                                for k in range(KT):
                                    nc.tensor.matmul(
                                        ps[:, :],
                                        w_sb[:, k, fofs:fofs + D],
                                        xh[k // 8][:, k % 8, :],
                                        start=(k == 0), stop=(k == KT - 1))
                                qb = ropep.tile([128, CH], f32r, tag="qb")
                                col = which * HPC + h
                                nc.scalar.activation(qb[:, :], ps[:, :], Identity,
                                                     bias=bqk_sb[:, col:col + 1])
                                qr_ps = psb.tile([128, CH], f32, tag="big")
                                nc.tensor.matmul(qr_ps[:, :], rot_sb[:, :], qb[:, :],
                                                 start=True, stop=True)
                                ct, st = tsl[2 * which], tsl[2 * which + 1]
                                t1 = ropep.tile([128, CH], f32, tag="t1", bufs=2)
                                nc.vector.tensor_mul(t1[:, :], qb[:, :], ct[:, :])
                                t2 = ropep.tile([128, CH], f32, tag="t2", bufs=2)
                                nc.vector.tensor_mul(t2[:, :], qr_ps[:, :], st[:, :])
                                if which == 0:
                                    nc.vector.tensor_add(dst[h][:, :], t1[:, :], t2[:, :])
                                else:
                                    cc = slice(c * CH, (c + 1) * CH)
                                    nc.vector.tensor_add(dst[h][:, cc], t1[:, :], t2[:, :])

                        for tsub in range(4):
                            tt = c * 4 + tsub
                            ps = psb.tile([128, FPC], f32, tag="big")
                            for k in range(KT):
                                nc.tensor.matmul(
                                    ps[:, :],
                                    xh[k // 8][:, k % 8, tsub * 128:tsub * 128 + 128],
                                    w_sb[:, k, 2 * FPC:3 * FPC],
                                    start=(k == 0), stop=False)
                            nc.tensor.matmul(ps[:, :], onesr_sb[:, :], bv_sb[:, :],
                                             start=False, stop=True)
                            nc.vector.tensor_copy(v_sb[:, tt, :], ps[:, :])

                        # ---- stage B: attention for q-chunk c, both heads ----
                        ot_c = [otp.tile([128, CH], f32r, tag=f"ot{h}", name=f"otc{h}",
                                         bufs=2) for h in range(HPC)]
                        cc = slice(c * CH, (c + 1) * CH)
                        njt = 4 * c + 4
                        for h in range(HPC):
                            o_ps = psb.tile([128, CH], f32, tag="big")
                            rs = pss.tile([1, CH], f32, tag="rs")
                            for j in range(njt):
                                s_ps = psb.tile([128, CH], f32, tag="big")
                                nc.tensor.matmul(
                                    s_ps[:, :],
                                    kt_sb[h][:, j * 128:j * 128 + 128],
                                    qt_sb[h][:, :],
                                    start=True, stop=True)
                                pt = ptp.tile([128, CH], f32r, tag="pt")
                                nc.scalar.activation(pt[:, :], s_ps[:, :], Exp)
                                if j >= 4 * c:
                                    nc.vector.tensor_mul(
                                        pt[:, :], pt[:, :],
                                        mask_sb[:, j - 4 * c, :])
                                nc.tensor.matmul(
                                    o_ps[:, :],
                                    v_sb[:, j, h * D:h * D + D],
                                    pt[:, :],
                                    start=(j == 0), stop=(j == njt - 1),
                                    skip_group_check=True)
                                nc.tensor.matmul(
                                    rs[:, :], onesc_sb[:, :], pt[:, :],
                                    start=(j == 0), stop=(j == njt - 1),
                                    skip_group_check=True)
                            r0 = smallp.tile([1, CH], f32, tag="r0")
                            nc.vector.reciprocal(r0[:, :], rs[:, :])
                            tn = smallp.tile([1, CH], f32, tag="tn")
                            nc.vector.tensor_mul(tn[:, :], rs[:, :], r0[:, :])
                            nc.vector.tensor_scalar(tn[:, :], tn[:, :], -1.0, 2.0,
                                                    Mult, Add)
                            r1 = smallp.tile([1, CH], f32r, tag="r1")
                            nc.vector.tensor_mul(r1[:, :], r0[:, :], tn[:, :])
                            bc_ps = psb.tile([128, CH], f32, tag="big")
                            nc.tensor.matmul(bc_ps[:, :], onesr_sb[:, :], r1[:, :],
                                             start=True, stop=True)
                            bc_sb = ybufp.tile([128, CH], f32, tag="bc", bufs=1)
                            nc.scalar.activation(bc_sb[:, :], bc_ps[:, :], Identity)
                            nc.vector.tensor_mul(ot_c[h][:, :], o_ps[:, :], bc_sb[:, :])

                        # ---- stage C: output projection for t-range of chunk c ----
                        for tloc in range(4):
                            ti = 4 * c + tloc
                            for gc in range(NCH):
                                yp = psb.tile([128, CH], f32, tag="big")
                                for h in range(HPC):
                                    nc.tensor.matmul(
                                        yp[:, :],
                                        ot_c[h][:, tloc * 128:tloc * 128 + 128],
                                        wo_sb[:, h, gc * CH:(gc + 1) * CH],
                                        start=(h == 0), stop=(h == HPC - 1))
                                yb = ybufp.tile([128, CH], f32, tag="yb", bufs=3)
                                nc.any.tensor_copy(yb[:, :], yp[:, :])
                                nc.sync.dma_start(
                                    out=y.ap()[b][ti * 128:ti * 128 + 128,
                                                  gc * CH:(gc + 1) * CH],
                                    in_=yb[:, :])

            if reps == 1:
                body()
            else:
                with tc.For_i(0, reps, 1):
                    body()

    nc.compile()
    return nc


def host_inputs(x, Wq, bq, Wk, bk, Wv, bv, Wo, bo):
    """Prepare per-core input maps from the full problem inputs."""
    x = np.asarray(x, np.float32)
    xTr = round_fp32r(np.ascontiguousarray(x.transpose(0, 2, 1)))

    # RoPE tables, 1-indexed positions, 1/sqrt(D) folded into the Q tables
    j = np.arange(D // 2, dtype=np.float64)
    thetas = ROPE_BASE ** (-2.0 * j / D)
    m = np.arange(1, T + 1, dtype=np.float64)
    ang = m[:, None] * thetas[None, :]          # [T, D/2]
    ang = np.concatenate([ang, ang], axis=1)    # [T, D]
    s = 1.0 / np.sqrt(D)
    tabs = np.stack([
        (np.cos(ang) * s).T, (np.sin(ang) * s).T,
        np.cos(ang).T, np.sin(ang).T,
    ]).astype(np.float32)                        # [4, D, T]

    # causal masks for the 4 diagonal alignments: mask_p[kk, qq] = qq >= 128p + kk
    kk = np.arange(128)[:, None]
    qq = np.arange(CH)[None, :]
    masks = np.stack([(qq >= 128 * p + kk) for p in range(4)]).astype(np.float32)

    onesr = np.ones((1, 128), np.float32)
    onesc = np.ones((128, 1), np.float32)
    rotm = np.zeros((D, D), np.float32)
    for d in range(D // 2):
        rotm[d + D // 2, d] = -1.0   # qrot[d] = -q[d+64]
        rotm[d, d + D // 2] = 1.0    # qrot[d+64] = q[d]

    in_maps = []
    for c in range(N_CORES):
        fs = slice(c * FPC, (c + 1) * FPC)
        wqkvT = np.concatenate([Wq[fs].T, Wk[fs].T, Wv[fs].T], axis=1)  # [E, 768]
        woT = np.ascontiguousarray(Wo[:, fs].T)                        # [256, E]
        bqk_cols = np.stack([
            bq[fs][:D], bq[fs][D:], bk[fs][:D], bk[fs][D:],
        ], axis=1).astype(np.float32)                                  # [128, 4]
        in_maps.append({
            "xT": xTr,
            "wqkvT": round_fp32r(np.ascontiguousarray(wqkvT)),
            "woT": round_fp32r(woT),
            "tabs": tabs,
            "masks": masks,
            "bqk": bqk_cols,
            "bvT": round_fp32r(np.asarray(bv[fs], np.float32)[None, :]),
            "ones_row": onesr,
            "ones_col": onesc,
            "rotm": rotm,
        })
    return in_maps


_NC_CACHE = {}


def get_nc(reps: int = 1):
    if reps not in _NC_CACHE:
        _NC_CACHE[reps] = build_nc(reps)
    return _NC_CACHE[reps]


def kernel(x, Wq, bq, Wk, bk, Wv, bv, Wo, bo):
    in_maps = host_inputs(x, Wq, bq, Wk, bk, Wv, bv, Wo, bo)
    nc = get_nc(1)
    res = run_bass_kernel_spmd(nc, in_maps, list(range(N_CORES)))
    out = np.zeros((B, T, E), np.float64)
    for c in range(N_CORES):
        out += res.results[c]["y"].astype(np.float64)
    out += np.asarray(bo, np.float64)[None, None, :]
    return out.astype(np.float32)


# revision 22
# speedup vs baseline: 2.2322x; 1.3604x over previous
"""Causal self-attention (B=2, T=2048, E=2048, H=16, D=128) on 8 NeuronCores.

Sharding: tensor-parallel over heads — each core owns 2 heads (256 features).
Per core: QKV projections for its head slice (fp32r matmuls), RoPE, causal
attention computed in S^T layout (keys on partitions), and a partial output
projection against its Wo row-slice. The host sums the 8 partials and adds bo.

All matmul operands are fp32r (fp32 rounded to 11 mantissa bits) — full PE
rate for moving free-dim >= 256, near-fp32 accuracy.
"""
import sys

sys.path.insert(0, "/opt/trn_rl_repo")

import numpy as np

import concourse.mybir as mybir
import concourse.tile as tile
from concourse import bacc
from concourse.bass_utils import run_bass_kernel_spmd

B, T, E, H = 2, 2048, 2048, 16
D = E // H            # 128 head dim
N_CORES = 8
HPC = H // N_CORES    # 2 heads per core
FPC = HPC * D         # 256 features per core
ROPE_BASE = 10000.0

CH = 512              # t-chunk (moving free dim)
NCH = T // CH         # 4 chunks
KT = E // 128         # 16 contraction tiles
NTT = T // 128        # 16 t-subtiles

f32 = mybir.dt.float32
f32r = mybir.dt.float32r


def round_fp32r(x: np.ndarray) -> np.ndarray:
    """Round fp32 to the fp32r representation (RNE to 11 mantissa bits)."""
    b = np.ascontiguousarray(x, np.float32).view(np.uint32).astype(np.uint64)
    r = b + 0x7FF + ((b >> 12) & 1)
    r = (r & ~np.uint64(0xFFF)).astype(np.uint32)
    return r.view(np.float32)


def build_nc(reps: int = 1):
    """Build the per-core Bass program. reps>1 wraps the body in a hardware
    repeat loop (identical work each iteration) for slope-timing."""
    nc = bacc.Bacc("TRN2", target_bir_lowering=False, debug=False,
                   num_devices=N_CORES)

    def mark(label):
        PHASE_MARKS.append((label, int(nc.get_next_instruction_name()[2:])))

    xT = nc.dram_tensor("xT", [B, E, T], f32r, kind="ExternalInput")
    wqkvT = nc.dram_tensor("wqkvT", [E, 3 * FPC], f32r, kind="ExternalInput")
    woT = nc.dram_tensor("woT", [FPC, E], f32r, kind="ExternalInput")
    tabs = nc.dram_tensor("tabs", [4, D, T], f32, kind="ExternalInput")
    masks = nc.dram_tensor("masks", [4, 128, CH], mybir.dt.bfloat16, kind="ExternalInput")
    bqk = nc.dram_tensor("bqk", [128, 2 * HPC], f32, kind="ExternalInput")
    bvT = nc.dram_tensor("bvT", [1, FPC], f32r, kind="ExternalInput")
    ones_row = nc.dram_tensor("ones_row", [1, 128], f32r, kind="ExternalInput")
    ones_col = nc.dram_tensor("ones_col", [128, 1], f32r, kind="ExternalInput")
    rotm = nc.dram_tensor("rotm", [D, D], f32r, kind="ExternalInput")
    y = nc.dram_tensor("y", [B, T, E], f32, kind="ExternalOutput")

    Exp = mybir.ActivationFunctionType.Exp
    Identity = mybir.ActivationFunctionType.Identity
    Mult = mybir.AluOpType.mult
    Add = mybir.AluOpType.add

    with tile.TileContext(nc) as tc:
        with (
            nc.allow_low_precision(reason="fp32r matmul operands are intentional"),
            tc.tile_pool(name="wpool", bufs=1) as wpool,
            tc.tile_pool(name="xc", bufs=2) as xcp,
            tc.tile_pool(name="qkv", bufs=1) as qkvp,
            tc.tile_pool(name="tab", bufs=1) as tabp,
            tc.tile_pool(name="const", bufs=1) as constp,
            tc.tile_pool(name="pt", bufs=4) as ptp,
            tc.tile_pool(name="ot", bufs=1) as otp,
            tc.tile_pool(name="ybuf", bufs=2) as ybufp,
            tc.tile_pool(name="rope", bufs=2) as ropep,
            tc.tile_pool(name="small", bufs=1) as smallp,
            tc.tile_pool(name="dramp", bufs=2, space="DRAM") as dramp,
            tc.tile_pool(name="psb", bufs=6, space="PSUM") as psb,
            tc.tile_pool(name="pss", bufs=1, space="PSUM") as pss,
        ):
            # ---- resident constants (loaded once) ----
            _wsrc = wqkvT.ap().rearrange("(kt p) f -> p kt f", p=128)
            w_parts = []
            for _wi, _wn in enumerate(("wq", "wk", "wv")):
                wp = wpool.tile([128, KT, FPC], f32r, tag=_wn, name=_wn)
                w_parts.append(wp)
            nc.sync.dma_start(out=w_parts[0][:, :, :], in_=_wsrc[:, :, 0:FPC])
            wo_sb = wpool.tile([128, HPC, E], f32r, tag="wo")
            mask_sb = constp.tile([128, 4, CH], mybir.dt.bfloat16, tag="masks")
            nc.sync.dma_start(out=mask_sb[:, :, :], in_=masks.ap().rearrange("m p q -> p m q"))
            bqk_sb = constp.tile([128, 2 * HPC], f32, tag="bqk")
            nc.sync.dma_start(out=bqk_sb[:, :], in_=bqk.ap())
            bv_sb = constp.tile([1, FPC], f32r, tag="bv")
            nc.sync.dma_start(out=bv_sb[:, :], in_=bvT.ap())
            onesr_sb = constp.tile([1, 128], f32r, tag="onesr")
            nc.sync.dma_start(out=onesr_sb[:, :], in_=ones_row.ap())
            onesc_sb = constp.tile([128, 1], f32r, tag="onesc")
            nc.sync.dma_start(out=onesc_sb[:, :], in_=ones_col.ap())
            rot_sb = constp.tile([D, D], f32r, tag="rotm")
            nc.sync.dma_start(out=rot_sb[:, :], in_=rotm.ap())

            def body():
                for b in range(B):
                    kt_sb = [qkvp.tile([128, T], f32r, tag=f"kt{h}", name=f"ktt{h}") for h in range(HPC)]
                    v_sb = qkvp.tile([128, NTT, FPC], f32r, tag="v")
                    def load_chunk(c):
                        xh = []
                        for half in range(2):
                            t_ = xcp.tile([128, 8, CH], f32r, tag="xc", name="xch")
                            src_ = xT.ap()[b][:, c * CH:(c + 1) * CH].rearrange(
                                "(et p) t -> p et t", p=128)
                            nc.sync.dma_start(out=t_[:, :, :],
                                              in_=src_[:, half * 8:half * 8 + 8, :])
                            xh.append(t_)
                        tsl = []
                        for ti in range(4):
                            tt = tabp.tile([128, CH], f32, tag=f"tab{ti}",
                                           name=f"tab{ti}", bufs=2)
                            nc.sync.dma_start(
                                out=tt[:, :],
                                in_=tabs.ap()[ti][:, c * CH:(c + 1) * CH])
                            tsl.append(tt)
                        return xh, tsl

                    nxt = load_chunk(0)
                    for c in range(NCH):
                        # ---- stage A: projections for chunk c ----
                        qt_sb = [qkvp.tile([128, CH], f32r, tag=f"qt{h}", name=f"qt{h}",
                                           bufs=2) for h in range(HPC)]
                        xh, tsl = nxt

                        mark(f"b{b}c{c}:A.x")
                        for which, dst in ((0, qt_sb), (1, kt_sb)):
                            if b == 0 and c == 0:
                                nc.sync.dma_start(
                                    out=w_parts[which + 1][:, :, :],
                                    in_=_wsrc[:, :, (which + 1) * FPC:(which + 2) * FPC])
                            for h in range(HPC):
                                fofs = h * D
                                ps = psb.tile([128, CH], f32, tag="big")
                                for k in range(KT):
                                    nc.tensor.matmul(
                                        ps[:, :],
                                        w_parts[which][:, k, fofs:fofs + D],
                                        xh[k // 8][:, k % 8, :],
                                        start=(k == 0), stop=(k == KT - 1))
                                qb = ropep.tile([128, CH], f32r, tag="qb")
                                col = which * HPC + h
                                nc.scalar.activation(qb[:, :], ps[:, :], Identity,
                                                     bias=bqk_sb[:, col:col + 1])
                                qr_ps = psb.tile([128, CH], f32, tag="big")
                                nc.tensor.matmul(qr_ps[:, :], rot_sb[:, :], qb[:, :],
                                                 start=True, stop=True)
                                ct, st = tsl[2 * which], tsl[2 * which + 1]
                                t1 = ropep.tile([128, CH], f32, tag="t1", bufs=2)
                                nc.vector.tensor_mul(t1[:, :], qb[:, :], ct[:, :])
                                t2 = ropep.tile([128, CH], f32, tag="t2", bufs=2)
                                nc.vector.tensor_mul(t2[:, :], qr_ps[:, :], st[:, :])
                                if which == 0:
                                    nc.vector.tensor_add(dst[h][:, :], t1[:, :], t2[:, :])
                                else:
                                    cc = slice(c * CH, (c + 1) * CH)
                                    nc.vector.tensor_add(dst[h][:, cc], t1[:, :], t2[:, :])

                        mark(f"b{b}c{c}:A.qk")
                        for tsub in range(4):
                            tt = c * 4 + tsub
                            ps = psb.tile([128, FPC], f32, tag="big")
                            for k in range(KT):
                                nc.tensor.matmul(
                                    ps[:, :],
                                    xh[k // 8][:, k % 8, tsub * 128:tsub * 128 + 128],
                                    w_parts[2][:, k, :],
                                    start=(k == 0), stop=False)
                            nc.tensor.matmul(ps[:, :], onesr_sb[:, :], bv_sb[:, :],
                                             start=False, stop=True)
                            nc.vector.tensor_copy(v_sb[:, tt, :], ps[:, :])

                        # ---- stage B: attention for q-chunk c, both heads ----
                        if b == 0 and c == 0:
                            nc.sync.dma_start(
                                out=wo_sb[:, :, :],
                                in_=woT.ap().rearrange("(ft p) g -> p ft g", p=128))
                        if c + 1 < NCH:
                            nxt = load_chunk(c + 1)
                        mark(f"b{b}c{c}:A.v")
                        ot_c = [otp.tile([128, CH], f32r, tag=f"ot{h}", name=f"otc{h}",
                                         bufs=2) for h in range(HPC)]
                        cc = slice(c * CH, (c + 1) * CH)
                        njt = 4 * c + 4
                        o_ps = {}
                        rs_t = [pss.tile([1, CH], f32, tag="rs", name=f"rs{h}",
                                         bufs=2) for h in range(HPC)]
                        pts = {}

                        def emit_s(h, j):
                            s_ps = psb.tile([128, CH], f32, tag="big",
                                            name="s_ps")
                            nc.tensor.matmul(
                                s_ps[:, :],
                                kt_sb[h][:, j * 128:j * 128 + 128],
                                qt_sb[h][:, :],
                                start=True, stop=True)
                            pt = ptp.tile([128, CH], f32r, tag="pt", name="pt")
                            nc.scalar.activation(pt[:, :], s_ps[:, :], Exp)
                            if j >= 4 * c:
                                nc.vector.tensor_mul(
                                    pt[:, :], pt[:, :],
                                    mask_sb[:, j - 4 * c, :])
                            pts[(h, j)] = pt

                        for h in range(HPC):
                            o_ps[h] = psb.tile([128, CH], f32, tag="big",
                                               name="o_ps")
                        for jj in range(min(2, njt)):
                            for h in range(HPC):
                                emit_s(h, jj)
                        for j in range(njt):
                            for h in range(HPC):
                                pt = pts.pop((h, j))
                                nc.tensor.matmul(
                                    o_ps[h][:, :],
                                    v_sb[:, j, h * D:h * D + D],
                                    pt[:, :],
                                    start=(j == 0), stop=(j == njt - 1),
                                    skip_group_check=True)
                                nc.tensor.matmul(
                                    rs_t[h][:, :],
                                    onesc_sb[:, :], pt[:, :],
                                    start=(j == 0), stop=(j == njt - 1),
                                    skip_group_check=True)
                            if j + 2 < njt:
                                for h in range(HPC):
                                    emit_s(h, j + 2)
                        for h in range(HPC):
                            rsl = rs_t[h][:, :]
                            r0 = smallp.tile([1, CH], f32, tag="r0")
                            nc.vector.reciprocal(r0[:, :], rsl)
                            tn = smallp.tile([1, CH], f32, tag="tn")
                            nc.vector.tensor_mul(tn[:, :], rsl, r0[:, :])
                            nc.vector.tensor_scalar(tn[:, :], tn[:, :], -1.0, 2.0,
                                                    Mult, Add)
                            r1r = smallp.tile([1, CH], f32r, tag="r1r")
                            nc.vector.tensor_mul(r1r[:, :], r0[:, :], tn[:, :])
                            bc_ps = psb.tile([128, CH], f32, tag="big")
                            nc.tensor.matmul(bc_ps[:, :], onesr_sb[:, :], r1r[:, :],
                                             start=True, stop=True)
                            bc_sb = ybufp.tile([128, CH], f32, tag="bc", bufs=2)
                            nc.scalar.activation(bc_sb[:, :], bc_ps[:, :], Identity)
                            nc.vector.tensor_mul(ot_c[h][:, :], o_ps[h][:, :], bc_sb[:, :])

                        # ---- stage C: output projection for t-range of chunk c ----
                        mark(f"b{b}c{c}:B")
                        for tloc in range(4):
                            ti = 4 * c + tloc
                            for gc in range(NCH):
                                yp = psb.tile([128, CH], f32, tag="big")
                                for h in range(HPC):
                                    nc.tensor.matmul(
                                        yp[:, :],
                                        ot_c[h][:, tloc * 128:tloc * 128 + 128],
                                        wo_sb[:, h, gc * CH:(gc + 1) * CH],
                                        start=(h == 0), stop=(h == HPC - 1))
                                yb = ybufp.tile([128, CH], f32, tag="yb", bufs=4)
                                if (tloc * NCH + gc) % 2 == 0:
                                    nc.scalar.activation(yb[:, :], yp[:, :], Identity)
                                    eng = nc.scalar
                                else:
                                    nc.vector.tensor_copy(yb[:, :], yp[:, :])
                                    eng = nc.sync
                                eng.dma_start(
                                    out=y.ap()[b][ti * 128:ti * 128 + 128,
                                                  gc * CH:(gc + 1) * CH],
                                    in_=yb[:, :])

            if reps == 1:
                body()
                mark("end")
            else:
                with tc.For_i(0, reps, 1):
                    body()

    nc.compile()
    return nc


def host_inputs(x, Wq, bq, Wk, bk, Wv, bv, Wo, bo):
    """Prepare per-core input maps from the full problem inputs."""
    x = np.asarray(x, np.float32)
    xTr = round_fp32r(np.ascontiguousarray(x.transpose(0, 2, 1)))

    # RoPE tables, 1-indexed positions, 1/sqrt(D) folded into the Q tables
    j = np.arange(D // 2, dtype=np.float64)
    thetas = ROPE_BASE ** (-2.0 * j / D)
    m = np.arange(1, T + 1, dtype=np.float64)
    ang = m[:, None] * thetas[None, :]          # [T, D/2]
    ang = np.concatenate([ang, ang], axis=1)    # [T, D]
    s = 1.0 / np.sqrt(D)
    tabs = np.stack([
        (np.cos(ang) * s).T, (np.sin(ang) * s).T,
        np.cos(ang).T, np.sin(ang).T,
    ]).astype(np.float32)                        # [4, D, T]

    # causal masks for the 4 diagonal alignments: mask_p[kk, qq] = qq >= 128p + kk
    kk = np.arange(128)[:, None]
    qq = np.arange(CH)[None, :]
    import ml_dtypes
    masks = np.stack([(qq >= 128 * p + kk) for p in range(4)]).astype(ml_dtypes.bfloat16)

    onesr = np.ones((1, 128), np.float32)
    onesc = np.ones((128, 1), np.float32)
    rotm = np.zeros((D, D), np.float32)
    for d in range(D // 2):
        rotm[d + D // 2, d] = -1.0   # qrot[d] = -q[d+64]
        rotm[d, d + D // 2] = 1.0    # qrot[d+64] = q[d]

    in_maps = []
    for c in range(N_CORES):
        fs = slice(c * FPC, (c + 1) * FPC)
        wqkvT = np.concatenate([Wq[fs].T, Wk[fs].T, Wv[fs].T], axis=1)  # [E, 768]
        woT = np.ascontiguousarray(Wo[:, fs].T)                        # [256, E]
        bqk_cols = np.stack([
            bq[fs][:D], bq[fs][D:], bk[fs][:D], bk[fs][D:],
        ], axis=1).astype(np.float32)                                  # [128, 4]
        in_maps.append({
            "xT": xTr,
            "wqkvT": round_fp32r(np.ascontiguousarray(wqkvT)),
            "woT": round_fp32r(woT),
            "tabs": tabs,
            "masks": masks,
            "bqk": bqk_cols,
            "bvT": round_fp32r(np.asarray(bv[fs], np.float32)[None, :]),
            "ones_row": onesr,
            "ones_col": onesc,
            "rotm": rotm,
        })
    return in_maps


PHASE_MARKS = []


_NC_CACHE = {}


def get_nc(reps: int = 1):
    if reps not in _NC_CACHE:
        _NC_CACHE[reps] = build_nc(reps)
    return _NC_CACHE[reps]


def kernel(x, Wq, bq, Wk, bk, Wv, bv, Wo, bo):
    in_maps = host_inputs(x, Wq, bq, Wk, bk, Wv, bv, Wo, bo)
    nc = get_nc(1)
    res = run_bass_kernel_spmd(nc, in_maps, list(range(N_CORES)))
    out = np.zeros((B, T, E), np.float64)
    for c in range(N_CORES):
        out += res.results[c]["y"].astype(np.float64)
    out += np.asarray(bo, np.float64)[None, None, :]
    return out.astype(np.float32)
